# revision 39
# baseline (speedup 1.0000x reference)
"""Multi-head self-attention (B=4, N=2048, D=1024, H=16) on 8 Trainium2 cores.

Sharding: batch (4) x head-group (2 groups of 8 heads) -> 8 cores.
Host-side shard prep also lays out the per-core inputs for the device:
x is passed transposed in bf16 (x^T [D, N]) and the w slice in bf16
[KO, 128, 3*GW], so the kernel does no on-device casts/transposes of inputs.

Each core computes, for its batch b and heads [8g, 8g+8):
  qkv = x_b @ w_slice            (projection, bf16 matmuls, fp32 accum)
  S^T[n,m] = K Q^T               (scores transposed: keys on partitions,
                                  head pair row-tiled K=64: the two heads'
                                  matmuls run concurrently in disjoint
                                  64-row strips of the PE array)
  E = exp(S^T / 8)               (ScalarE, bf16; max score ~8 so exp is
                                  safe without max-subtraction)
  out^T[d,m], den[m] = [V|1]^T E (bf16 matmul per key chunk, accumulated)
  out = transpose(out^T) / den   (bf16 SBUF->SBUF xbar-transpose DMA, then
                                  DVE reciprocal+mul; PE transposes only for
                                  the final group where the PE is idle)

Phases:
  A: load x^T and w straight into SBUF; kT, V, qT projection chains.
  B: per (head-pair, m-tile): scores and AV in 2-key-chunk groups (hides PE
     drain between row-tiled pairs), exp at the ScalarE floor, DMA-transposed
     output path.

Device layouts:
  qT, kT [128, 4, 2048] bf16     : chunk hp holds head 2hp on partitions 0-63
                                   and head 2hp+1 on partitions 64-127
  v_sb [128, 16, 8, 80] bf16     : [key-in-chunk, chunk, head,
                                    head_dim | ones@64 | zero pad to 80]
"""

import numpy as np
import ml_dtypes

import concourse.bacc as bacc
import concourse.bass_utils as bass_utils
import concourse.mybir as mybir
import concourse.tile as tile
from concourse.masks import make_identity

B, N, D = 4, 2048, 1024
H, HD = 16, 64
NCORES = 8
HPC = 8  # heads per core
GW = HPC * HD  # 512, output-column group width per core
P = 128
KO = D // P  # 8 k-chunks of 128
HPAIRS = HPC // 2  # 4 head pairs
VP = 128  # V row pitch: 64 dims + ones col + pad to 128 so AV LDWEIGHTS gets FWL

F32 = mybir.dt.float32
BF16 = mybir.dt.bfloat16
EXPF = mybir.ActivationFunctionType.Exp

_CACHE: dict = {}


def _emit(nc, tc, x_d, wkq_d, wv_d, o_d, n=N):
    MT = n // 512
    NCH = n // P
    NPC = NCH // 2  # chunk pairs

    with (
        tc.tile_pool(name="constp", bufs=1) as constp,
        tc.tile_pool(name="xp", bufs=1) as xp,
        tc.tile_pool(name="qkp", bufs=1) as qkp,
        tc.tile_pool(name="vp", bufs=1) as vp,
        tc.tile_pool(name="wp", bufs=1) as wp,
    ):
        wuc = constp.tile([P, P], BF16)
        nc.vector.memset(wuc, 0.5)
        # preload the exp ACT table during the DMA lead (walrus emits the
        # table load before the first ACTIVATE)
        wue = constp.tile([P, 1], BF16)
        nc.scalar.activation(wue, wuc[:, 0:1], EXPF, scale=1.0)
        ident = constp.tile([P, P], BF16)
        make_identity(nc, ident)
        qT = qkp.tile([P, HPAIRS, n], BF16)
        kT = qkp.tile([P, HPAIRS, n], BF16)
        v_sb = vp.tile([P, NCH, HPC, VP], BF16)
        ones_c = constp.tile([P, 1], F32)
        nc.vector.memset(ones_c, 1.0)
        nc.gpsimd.memset(v_sb, 0.0)
        nc.gpsimd.tensor_copy(
            v_sb[:, :, :, HD], ones_c.to_broadcast([P, NCH, HPC])
        )

        w_b = wp.tile([P, KO, 3 * GW], BF16)
        xT = xp.tile([P, KO, n], BF16)

        # direct loads (already bf16 / pre-transposed on host), ordered so
        # the first attention group starts after ~2MB: x column-quarters
        # (first quarter gates kT/qT of m-tile 0), w in k, q, v sections
        # quarter 0 per-ko so the first kT chain streams behind the loads
        for ko in range(KO):
            nc.sync.dma_start(xT[:, ko, 0:512], x_d[ko][:, 0:512])
        # w-v on the sync ring right after x quarter 0, so the scalar ring
        # finishes the kq weights early and V chains can weave from ~8us
        nc.sync.dma_start(
            w_b[:, :, 2 * GW : 3 * GW], wv_d.rearrange("ko p c -> p ko c")
        )
        for q in range(1, 4):
            nc.sync.dma_start(
                xT[:, :, q * 512 : (q + 1) * 512],
                x_d.rearrange("ko p n -> p ko n")[:, :, q * 512 : (q + 1) * 512],
            )
        nc.scalar.dma_start(
            w_b[:, :, 0 : 2 * GW], wkq_d.rearrange("ko p c -> p ko c")
        )

        # ---- Fused projection + attention ----
        with (
            tc.tile_pool(name="psA", bufs=1, space="PSUM") as psA,
            tc.tile_pool(name="ep", bufs=6) as ep,
            tc.tile_pool(name="otp", bufs=4) as otp,
            tc.tile_pool(name="otTp", bufs=4) as otTp,
            tc.tile_pool(name="op", bufs=4) as op,
            tc.tile_pool(name="rp", bufs=8) as rp,
            tc.tile_pool(name="psS", bufs=2, space="PSUM") as psS,
            tc.tile_pool(name="psO", bufs=3, space="PSUM") as psO,
        ):

            def proj_chain(dst, wsrc, col0, mres):
                ps = psA.tile([P, 512], F32, tag="psA", name="ps")
                for ko in range(KO):
                    nc.tensor.matmul(
                        ps,
                        lhsT=wsrc[:, ko, col0 : col0 + P],
                        rhs=xT[:, ko, mres],
                        start=(ko == 0),
                        stop=(ko == KO - 1),
                    )
                nc.vector.tensor_copy(dst, ps)

            def emit_kT(hp, mt):
                mres = slice(mt * 512, (mt + 1) * 512)
                proj_chain(kT[:, hp, mres], w_b, hp * P, mres)

            def emit_qT(hp, mt):
                mres = slice(mt * 512, (mt + 1) * 512)
                proj_chain(qT[:, hp, mres], w_b, GW + hp * P, mres)

            def emit_V(nch):
                psv = psA.tile([P, GW], F32, tag="psA", name="psv")
                for ko in range(KO):
                    nc.tensor.matmul(
                        psv,
                        lhsT=xT[:, ko, nch * P : (nch + 1) * P],
                        rhs=w_b[:, ko, 2 * GW : 3 * GW],
                        start=(ko == 0),
                        stop=(ko == KO - 1),
                    )
                nc.vector.tensor_copy(
                    v_sb[:, nch, :, 0:HD],
                    psv.rearrange("p (h d) -> p h d", d=HD),
                )

            def scores(hp, nch, mres):
                """Row-tiled concurrent score pair for one key chunk."""
                nres = slice(nch * P, (nch + 1) * P)
                pss = psS.tile([P, 1024], F32, tag="pss")
                nc.tensor.matmul(
                    pss[:, 0:512],
                    lhsT=kT[0:64, hp, nres],
                    rhs=qT[0:64, hp, mres],
                    start=True,
                    stop=True,
                )
                nc.tensor.matmul(
                    pss[:, 512:1024],
                    lhsT=kT[64:128, hp, nres],
                    rhs=qT[64:128, hp, mres],
                    start=True,
                    stop=True,
                )
                return pss

            def attention_hp(mt, hp, weave={}, last=False):
                # weave: list of zero-arg emitters, spread across pc groups
                mres = slice(mt * 512, (mt + 1) * 512)
                po0 = psO.tile([VP, 512], F32, tag="po", name="po0")
                po1 = psO.tile([VP, 512], F32, tag="po", name="po1")
                # software-pipelined: next pc's score pair is emitted before
                # this pc's AV matmuls so it completes during exp and the
                # ScalarE never waits on the PE.  weave: dict slot->emitters;
                # slot s items are emitted just before the scores of pc=s.
                for f in weave.get(0, ()):
                    f()
                pss_a = scores(hp, 0, mres)
                pss_b = scores(hp, 1, mres)
                for pc in range(NPC):
                    ca, cb = 2 * pc, 2 * pc + 1
                    e_a = ep.tile([P, 1024], BF16, tag="e", name="ea")
                    e_b = ep.tile([P, 1024], BF16, tag="e", name="eb")
                    nc.scalar.activation(e_a, pss_a, EXPF, scale=0.125)
                    nc.scalar.activation(e_b, pss_b, EXPF, scale=0.125)
                    if pc + 1 < NPC:
                        for f in weave.get(pc + 1, ()):
                            f()
                        pss_a = scores(hp, 2 * pc + 2, mres)
                        pss_b = scores(hp, 2 * pc + 3, mres)
                    for c, e in ((ca, e_a), (cb, e_b)):
                        nc.tensor.matmul(
                            po0,
                            lhsT=v_sb[:, c, 2 * hp],
                            rhs=e[:, 0:512],
                            start=(c == 0),
                            stop=(c == NCH - 1),
                        )
                        nc.tensor.matmul(
                            po1,
                            lhsT=v_sb[:, c, 2 * hp + 1],
                            rhs=e[:, 512:1024],
                            start=(c == 0),
                            stop=(c == NCH - 1),
                        )
                # out^T -> bf16 -> DRAM -> xbar transpose -> normalize -> out
                # (last group: PE transposes instead -- the PE is idle and the
                # DRAM round-trip latency would be fully exposed)
                o2g = op.tile([P, 4, P], F32, tag="o2g")
                for h01, po in ((0, po0), (1, po1)):
                    ot = otp.tile([VP, 512], BF16, tag="ot")
                    nc.vector.tensor_copy(ot, po)
                    if not last:
                        otT = otTp.tile([P, 4, VP], BF16, tag="otT")
                        nc.sync.dma_start_transpose(otT, ot)
                    for ms in range(4):
                        r = rp.tile([P, 1], F32, tag="r")
                        if last:
                            pt = psA.tile([P, VP], BF16, tag="psA", name="pt")
                            nc.tensor.transpose(
                                pt,
                                ot[:, ms * P : (ms + 1) * P],
                                ident[0:VP, 0:VP],
                            )
                            num, den = pt[:, 0:HD], pt[:, HD : HD + 1]
                        else:
                            num = otT[:, ms, 0:HD]
                            den = otT[:, ms, HD : HD + 1]
                        nc.vector.reciprocal(r, den)
                        nc.vector.tensor_mul(
                            out=o2g[:, ms, h01 * HD : (h01 + 1) * HD],
                            in0=num,
                            in1=r.to_broadcast([P, HD]),
                        )
                # one 256KB store for the whole group
                nc.sync.dma_start(
                    o_d[mt * 512 : (mt + 1) * 512, hp * P : (hp + 1) * P].rearrange(
                        "(ms p) c -> p ms c", p=P
                    ),
                    o2g,
                )

            # HAM warmup: ~3.5us of back-to-back dummy matmuls trips the
            # PE clock gate to 8/8 before the DMA-paced lead chains run
            wu = psA.tile([P, P], F32, tag="psA", name="wu")
            for _ in range(28):
                nc.tensor.matmul(
                    wu, lhsT=xT[:, 0, 0:P], rhs=xT[:, 0, 0:P], start=True, stop=True
                )

            # minimal lead: only what the first score pair needs
            emit_kT(0, 0)
            emit_qT(0, 0)

            def K(hp, mt):
                return lambda: emit_kT(hp, mt)

            def Q(hp, mt):
                return lambda: emit_qT(hp, mt)

            def V(c):
                return lambda: emit_V(c)

            def spread(items):
                # spread items over slots 1..7
                return {
                    sl: [f for i, f in enumerate(items) if 1 + i * 7 // len(items) == sl]
                    for sl in range(1, 8)
                }

            # weave schedules: group (0,0) is deadline-slotted (kT(0,m) before
            # the scores that read those key chunks, V(2s-2),V(2s-1) before
            # AV of pc=s-1); later groups just spread their chains
            W = {
                (0, 0): {
                    1: [V(0), V(1)],
                    2: [V(2), V(3), K(0, 1)],
                    3: [V(4), V(5), V(6)],
                    4: [V(7), V(8), K(0, 2)],
                    5: [V(9), V(10), V(11)],
                    6: [V(12), V(13), K(0, 3)],
                    7: [V(14), V(15), Q(0, 1)],
                },
                (0, 1): spread([Q(0, 2)]),
                (0, 2): spread([Q(0, 3)]),
                (0, 3): spread([K(1, 0), Q(1, 0)]),
                (1, 0): {2: [K(1, 1)], 4: [K(1, 2), Q(1, 1)], 6: [K(1, 3)]},
                (1, 1): spread([Q(1, 2)]),
                (1, 2): spread([Q(1, 3)]),
                (1, 3): spread([K(2, 0), Q(2, 0)]),
                (2, 0): {2: [K(2, 1)], 4: [K(2, 2), Q(2, 1)], 6: [K(2, 3)]},
                (2, 1): spread([Q(2, 2)]),
                (2, 2): spread([Q(2, 3)]),
                (2, 3): spread([K(3, 0), Q(3, 0)]),
                (3, 0): {2: [K(3, 1)], 4: [K(3, 2), Q(3, 1)], 6: [K(3, 3)]},
                (3, 1): spread([Q(3, 2)]),
                (3, 2): spread([Q(3, 3)]),
            }
            for hp in range(HPAIRS):
                for mt in range(MT):
                    attention_hp(
                        mt, hp, W.get((hp, mt), {}),
                        last=(hp == HPAIRS - 1 and mt == MT - 1),
                    )


def build(n=N, num_devices=NCORES, reps=1):
    key = (n, num_devices, reps)
    if key in _CACHE:
        return _CACHE[key]
    nc = bacc.Bacc("TRN2", target_bir_lowering=False, debug=False, num_devices=num_devices)
    x_d = nc.dram_tensor("x_s", [KO, P, n], BF16, kind="ExternalInput").ap()
    wkq_d = nc.dram_tensor("wkq_s", [KO, P, 2 * GW], BF16, kind="ExternalInput").ap()
    wv_d = nc.dram_tensor("wv_s", [KO, P, GW], BF16, kind="ExternalInput").ap()
    o_d = nc.dram_tensor("o_s", [n, GW], F32, kind="ExternalOutput").ap()
    with tile.TileContext(nc) as tc:
        for _ in range(reps):
            _emit(nc, tc, x_d, wkq_d, wv_d, o_d, n=n)
    nc.compile()
    _CACHE[key] = nc
    return nc


def make_in_maps(x, w_qkv):
    x = np.asarray(x, dtype=np.float32)
    w_qkv = np.asarray(w_qkv, dtype=np.float32)
    in_maps = []
    for c in range(NCORES):
        b, g = divmod(c, 2)
        xs = np.ascontiguousarray(x[b].T).astype(ml_dtypes.bfloat16).reshape(KO, P, N)
        wkq = (
            np.ascontiguousarray(
                np.concatenate(
                    [
                        w_qkv[:, D + g * GW : D + (g + 1) * GW],
                        w_qkv[:, g * GW : (g + 1) * GW],
                    ],
                    axis=1,
                )
            )
            .astype(ml_dtypes.bfloat16)
            .reshape(KO, P, 2 * GW)
        )
        wv = (
            np.ascontiguousarray(w_qkv[:, 2 * D + g * GW : 2 * D + (g + 1) * GW])
            .astype(ml_dtypes.bfloat16)
            .reshape(KO, P, GW)
        )
        in_maps.append({"x_s": xs, "wkq_s": wkq, "wv_s": wv})
    return in_maps


def assemble(results):
    out = np.empty((B, N, D), np.float32)
    for c in range(NCORES):
        b, g = divmod(c, 2)
        out[b][:, g * GW : (g + 1) * GW] = results[c]["o_s"]
    return out


def kernel(x, w_qkv, **run_kwargs):
    nc = build()
    in_maps = make_in_maps(x, w_qkv)
    res = bass_utils.run_bass_kernel_spmd(
        nc, in_maps, core_ids=list(range(NCORES)), **run_kwargs
    )
    out = assemble(res.results)
    if run_kwargs:
        kernel.last_result = res
    return out


# revision 40
# speedup vs baseline: 1.0550x; 1.0550x over previous
"""Multi-head self-attention (B=4, N=2048, D=1024, H=16) on 8 Trainium2 cores.

Sharding: batch (4) x head-group (2 groups of 8 heads) -> 8 cores.
Host-side shard prep also lays out the per-core inputs for the device:
x is passed transposed in bf16 (x^T [D, N]) and the w slice in bf16
[KO, 128, 3*GW], so the kernel does no on-device casts/transposes of inputs.

Each core computes, for its batch b and heads [8g, 8g+8):
  qkv = x_b @ w_slice            (projection, bf16 matmuls, fp32 accum)
  S^T[n,m] = K Q^T               (scores transposed: keys on partitions,
                                  head pair row-tiled K=64: the two heads'
                                  matmuls run concurrently in disjoint
                                  64-row strips of the PE array)
  E = exp(S^T / 8)               (ScalarE, bf16; max score ~8 so exp is
                                  safe without max-subtraction)
  out^T[d,m], den[m] = [V|1]^T E (bf16 matmul per key chunk, accumulated)
  out = transpose(out^T) / den   (bf16 SBUF->SBUF xbar-transpose DMA, then
                                  DVE reciprocal+mul; PE transposes only for
                                  the final group where the PE is idle)

Phases:
  A: load x^T and w straight into SBUF; kT, V, qT projection chains.
  B: per (head-pair, m-tile): scores and AV in 2-key-chunk groups (hides PE
     drain between row-tiled pairs), exp at the ScalarE floor, DMA-transposed
     output path.

Device layouts:
  qT, kT [128, 4, 2048] bf16     : chunk hp holds head 2hp on partitions 0-63
                                   and head 2hp+1 on partitions 64-127
  v_sb [128, 16, 8, 80] bf16     : [key-in-chunk, chunk, head,
                                    head_dim | ones@64 | zero pad to 80]
"""

import numpy as np
import ml_dtypes

import concourse.bacc as bacc
import concourse.bass_utils as bass_utils
import concourse.mybir as mybir
import concourse.tile as tile
from concourse.masks import make_identity

B, N, D = 4, 2048, 1024
H, HD = 16, 64
NCORES = 8
HPC = 8  # heads per core
GW = HPC * HD  # 512, output-column group width per core
P = 128
KO = D // P  # 8 k-chunks of 128
HPAIRS = HPC // 2  # 4 head pairs
VP = 128  # V row pitch: 64 dims + ones col + pad to 128 so AV LDWEIGHTS gets FWL

F32 = mybir.dt.float32
BF16 = mybir.dt.bfloat16
EXPF = mybir.ActivationFunctionType.Exp

_CACHE: dict = {}


def _emit(nc, tc, x_d, wkq_d, wv_d, o_d, n=N):
    MT = n // 512
    NCH = n // P
    NPC = NCH // 2  # chunk pairs

    with (
        tc.tile_pool(name="constp", bufs=1) as constp,
        tc.tile_pool(name="xp", bufs=1) as xp,
        tc.tile_pool(name="qkp", bufs=1) as qkp,
        tc.tile_pool(name="vp", bufs=1) as vp,
        tc.tile_pool(name="wp", bufs=1) as wp,
    ):
        wuc = constp.tile([P, P], BF16)
        nc.vector.memset(wuc, 0.5)
        # preload the exp ACT table during the DMA lead (walrus emits the
        # table load before the first ACTIVATE)
        wue = constp.tile([P, 1], BF16)
        nc.scalar.activation(wue, wuc[:, 0:1], EXPF, scale=1.0)
        ident = constp.tile([P, P], BF16)
        make_identity(nc, ident)
        qT = qkp.tile([P, HPAIRS, n], BF16)
        kT = qkp.tile([P, HPAIRS, n], BF16)
        v_sb = vp.tile([P, NCH, HPC, VP], BF16)
        ones_c = constp.tile([P, 1], F32)
        nc.vector.memset(ones_c, 1.0)
        nc.gpsimd.memset(v_sb, 0.0)
        nc.gpsimd.tensor_copy(
            v_sb[:, :, :, HD], ones_c.to_broadcast([P, NCH, HPC])
        )

        w_b = wp.tile([P, KO, 3 * GW], BF16)
        xT = xp.tile([P, KO, n], BF16)

        # direct loads (already bf16 / pre-transposed on host), ordered so
        # the first attention group starts after ~2MB: x column-quarters
        # (first quarter gates kT/qT of m-tile 0), w in k, q, v sections
        # quarter 0 per-ko so the first kT chain streams behind the loads
        for ko in range(KO):
            nc.sync.dma_start(xT[:, ko, 0:512], x_d[ko][:, 0:512])
        # w-v on the sync ring right after x quarter 0, so the scalar ring
        # finishes the kq weights early and V chains can weave from ~8us
        nc.sync.dma_start(
            w_b[:, :, 2 * GW : 3 * GW], wv_d.rearrange("ko p c -> p ko c")
        )
        for q in range(1, 4):
            nc.sync.dma_start(
                xT[:, :, q * 512 : (q + 1) * 512],
                x_d.rearrange("ko p n -> p ko n")[:, :, q * 512 : (q + 1) * 512],
            )
        nc.scalar.dma_start(
            w_b[:, :, 0 : 2 * GW], wkq_d.rearrange("ko p c -> p ko c")
        )

        # ---- Fused projection + attention ----
        with (
            tc.tile_pool(name="psA", bufs=2, space="PSUM") as psA,
            tc.tile_pool(name="ep", bufs=6) as ep,
            tc.tile_pool(name="otp", bufs=4) as otp,
            tc.tile_pool(name="otTp", bufs=4) as otTp,
            tc.tile_pool(name="op", bufs=4) as op,
            tc.tile_pool(name="rp", bufs=8) as rp,
            tc.tile_pool(name="psS", bufs=2, space="PSUM") as psS,
            tc.tile_pool(name="psO", bufs=2, space="PSUM") as psO,
        ):

            def proj_chain(dst, wsrc, col0, mres):
                ps = psA.tile([P, 512], F32, tag="psA", name="ps")
                for ko in range(KO):
                    nc.tensor.matmul(
                        ps,
                        lhsT=wsrc[:, ko, col0 : col0 + P],
                        rhs=xT[:, ko, mres],
                        start=(ko == 0),
                        stop=(ko == KO - 1),
                    )
                nc.vector.tensor_copy(dst, ps)

            def emit_kT(hp, mt):
                mres = slice(mt * 512, (mt + 1) * 512)
                proj_chain(kT[:, hp, mres], w_b, hp * P, mres)

            def emit_qT(hp, mt):
                mres = slice(mt * 512, (mt + 1) * 512)
                proj_chain(qT[:, hp, mres], w_b, GW + hp * P, mres)

            def emit_V(nch):
                psv = psA.tile([P, GW], F32, tag="psA", name="psv")
                for ko in range(KO):
                    nc.tensor.matmul(
                        psv,
                        lhsT=xT[:, ko, nch * P : (nch + 1) * P],
                        rhs=w_b[:, ko, 2 * GW : 3 * GW],
                        start=(ko == 0),
                        stop=(ko == KO - 1),
                    )
                nc.vector.tensor_copy(
                    v_sb[:, nch, :, 0:HD],
                    psv.rearrange("p (h d) -> p h d", d=HD),
                )

            def scores(hp, nch, mres):
                """Row-tiled concurrent score pair for one key chunk."""
                nres = slice(nch * P, (nch + 1) * P)
                pss = psS.tile([P, 1024], F32, tag="pss")
                nc.tensor.matmul(
                    pss[:, 0:512],
                    lhsT=kT[0:64, hp, nres],
                    rhs=qT[0:64, hp, mres],
                    start=True,
                    stop=True,
                )
                nc.tensor.matmul(
                    pss[:, 512:1024],
                    lhsT=kT[64:128, hp, nres],
                    rhs=qT[64:128, hp, mres],
                    start=True,
                    stop=True,
                )
                return pss

            def attention_hp(mt, hp, weave={}, last=False):
                # weave: list of zero-arg emitters, spread across pc groups
                mres = slice(mt * 512, (mt + 1) * 512)
                po0 = psO.tile([VP, 512], F32, tag="po", name="po0")
                po1 = psO.tile([VP, 512], F32, tag="po", name="po1")
                # software-pipelined: next pc's score pair is emitted before
                # this pc's AV matmuls so it completes during exp and the
                # ScalarE never waits on the PE.  weave: dict slot->emitters;
                # slot s items are emitted just before the scores of pc=s.
                for f in weave.get(0, ()):
                    f()
                pss_a = scores(hp, 0, mres)
                pss_b = scores(hp, 1, mres)
                for pc in range(NPC):
                    ca, cb = 2 * pc, 2 * pc + 1
                    e_a = ep.tile([P, 1024], BF16, tag="e", name="ea")
                    e_b = ep.tile([P, 1024], BF16, tag="e", name="eb")
                    nc.scalar.activation(e_a, pss_a, EXPF, scale=0.125)
                    nc.scalar.activation(e_b, pss_b, EXPF, scale=0.125)
                    if pc + 1 < NPC:
                        for f in weave.get(pc + 1, ()):
                            f()
                        pss_a = scores(hp, 2 * pc + 2, mres)
                        pss_b = scores(hp, 2 * pc + 3, mres)
                    for c, e in ((ca, e_a), (cb, e_b)):
                        nc.tensor.matmul(
                            po0,
                            lhsT=v_sb[:, c, 2 * hp],
                            rhs=e[:, 0:512],
                            start=(c == 0),
                            stop=(c == NCH - 1),
                        )
                        nc.tensor.matmul(
                            po1,
                            lhsT=v_sb[:, c, 2 * hp + 1],
                            rhs=e[:, 512:1024],
                            start=(c == 0),
                            stop=(c == NCH - 1),
                        )
                # out^T -> bf16 -> DRAM -> xbar transpose -> normalize -> out
                # (last group: PE transposes instead -- the PE is idle and the
                # DRAM round-trip latency would be fully exposed)
                o2g = op.tile([P, 4, P], F32, tag="o2g")
                for h01, po in ((0, po0), (1, po1)):
                    ot = otp.tile([VP, 512], BF16, tag="ot")
                    nc.vector.tensor_copy(ot, po)
                    if not last:
                        otT = otTp.tile([P, 4, VP], BF16, tag="otT")
                        nc.sync.dma_start_transpose(otT, ot)
                    for ms in range(4):
                        r = rp.tile([P, 1], F32, tag="r")
                        if last:
                            pt = psA.tile([P, VP], BF16, tag="psA", name="pt")
                            nc.tensor.transpose(
                                pt,
                                ot[:, ms * P : (ms + 1) * P],
                                ident[0:VP, 0:VP],
                            )
                            num, den = pt[:, 0:HD], pt[:, HD : HD + 1]
                        else:
                            num = otT[:, ms, 0:HD]
                            den = otT[:, ms, HD : HD + 1]
                        nc.vector.reciprocal(r, den)
                        nc.vector.tensor_mul(
                            out=o2g[:, ms, h01 * HD : (h01 + 1) * HD],
                            in0=num,
                            in1=r.to_broadcast([P, HD]),
                        )
                # one 256KB store for the whole group
                nc.sync.dma_start(
                    o_d[mt * 512 : (mt + 1) * 512, hp * P : (hp + 1) * P].rearrange(
                        "(ms p) c -> p ms c", p=P
                    ),
                    o2g,
                )

            # HAM warmup: ~3.5us of back-to-back dummy matmuls trips the
            # PE clock gate to 8/8 before the DMA-paced lead chains run
            wu = psA.tile([P, P], F32, tag="psA", name="wu")
            for _ in range(28):
                nc.tensor.matmul(
                    wu, lhsT=xT[:, 0, 0:P], rhs=xT[:, 0, 0:P], start=True, stop=True
                )

            # minimal lead: only what the first score pair needs
            emit_kT(0, 0)
            emit_qT(0, 0)

            def K(hp, mt):
                return lambda: emit_kT(hp, mt)

            def Q(hp, mt):
                return lambda: emit_qT(hp, mt)

            def V(c):
                return lambda: emit_V(c)

            def spread(items):
                # spread items over slots 1..7
                return {
                    sl: [f for i, f in enumerate(items) if 1 + i * 7 // len(items) == sl]
                    for sl in range(1, 8)
                }

            # weave schedules: group (0,0) is deadline-slotted (kT(0,m) before
            # the scores that read those key chunks, V(2s-2),V(2s-1) before
            # AV of pc=s-1); later groups just spread their chains
            W = {
                (0, 0): {
                    1: [V(0), V(1)],
                    2: [V(2), V(3), K(0, 1)],
                    3: [V(4), V(5), V(6)],
                    4: [V(7), V(8), K(0, 2)],
                    5: [V(9), V(10), V(11)],
                    6: [V(12), V(13), K(0, 3)],
                    7: [V(14), V(15), Q(0, 1)],
                },
                (0, 1): spread([Q(0, 2)]),
                (0, 2): spread([Q(0, 3)]),
                (0, 3): spread([K(1, 0), Q(1, 0)]),
                (1, 0): {2: [K(1, 1)], 4: [K(1, 2), Q(1, 1)], 6: [K(1, 3)]},
                (1, 1): spread([Q(1, 2)]),
                (1, 2): spread([Q(1, 3)]),
                (1, 3): spread([K(2, 0), Q(2, 0)]),
                (2, 0): {2: [K(2, 1)], 4: [K(2, 2), Q(2, 1)], 6: [K(2, 3)]},
                (2, 1): spread([Q(2, 2)]),
                (2, 2): spread([Q(2, 3)]),
                (2, 3): spread([K(3, 0), Q(3, 0)]),
                (3, 0): {2: [K(3, 1)], 4: [K(3, 2), Q(3, 1)], 6: [K(3, 3)]},
                (3, 1): spread([Q(3, 2)]),
                (3, 2): spread([Q(3, 3)]),
            }
            for hp in range(HPAIRS):
                for mt in range(MT):
                    attention_hp(
                        mt, hp, W.get((hp, mt), {}),
                        last=(hp == HPAIRS - 1 and mt == MT - 1),
                    )


def build(n=N, num_devices=NCORES, reps=1):
    key = (n, num_devices, reps)
    if key in _CACHE:
        return _CACHE[key]
    nc = bacc.Bacc("TRN2", target_bir_lowering=False, debug=False, num_devices=num_devices)
    x_d = nc.dram_tensor("x_s", [KO, P, n], BF16, kind="ExternalInput").ap()
    wkq_d = nc.dram_tensor("wkq_s", [KO, P, 2 * GW], BF16, kind="ExternalInput").ap()
    wv_d = nc.dram_tensor("wv_s", [KO, P, GW], BF16, kind="ExternalInput").ap()
    o_d = nc.dram_tensor("o_s", [n, GW], F32, kind="ExternalOutput").ap()
    with tile.TileContext(nc) as tc:
        for _ in range(reps):
            _emit(nc, tc, x_d, wkq_d, wv_d, o_d, n=n)
    nc.compile()
    _CACHE[key] = nc
    return nc


def make_in_maps(x, w_qkv):
    x = np.asarray(x, dtype=np.float32)
    w_qkv = np.asarray(w_qkv, dtype=np.float32)
    in_maps = []
    for c in range(NCORES):
        b, g = divmod(c, 2)
        xs = np.ascontiguousarray(x[b].T).astype(ml_dtypes.bfloat16).reshape(KO, P, N)
        wkq = (
            np.ascontiguousarray(
                np.concatenate(
                    [
                        w_qkv[:, D + g * GW : D + (g + 1) * GW],
                        w_qkv[:, g * GW : (g + 1) * GW],
                    ],
                    axis=1,
                )
            )
            .astype(ml_dtypes.bfloat16)
            .reshape(KO, P, 2 * GW)
        )
        wv = (
            np.ascontiguousarray(w_qkv[:, 2 * D + g * GW : 2 * D + (g + 1) * GW])
            .astype(ml_dtypes.bfloat16)
            .reshape(KO, P, GW)
        )
        in_maps.append({"x_s": xs, "wkq_s": wkq, "wv_s": wv})
    return in_maps


def assemble(results):
    out = np.empty((B, N, D), np.float32)
    for c in range(NCORES):
        b, g = divmod(c, 2)
        out[b][:, g * GW : (g + 1) * GW] = results[c]["o_s"]
    return out


def kernel(x, w_qkv, **run_kwargs):
    nc = build()
    in_maps = make_in_maps(x, w_qkv)
    res = bass_utils.run_bass_kernel_spmd(
        nc, in_maps, core_ids=list(range(NCORES)), **run_kwargs
    )
    out = assemble(res.results)
    if run_kwargs:
        kernel.last_result = res
    return out
